# revision 1
# baseline (speedup 1.0000x reference)
"""GAT message-passing model on 8 Trainium2 NeuronCores.

Strategy: edges sorted by destination node on the host; nodes split into 8
contiguous ranges balanced by incoming-edge count (one range per core).  Each
core computes the full k/v projection tables (replicated), its local q rows,
then processes its edge shard window-by-window: windows of <=128 contiguous
dst nodes with <=TPW*128 edges, padded to a fixed TPW tiles of 128 edge
slots, so all 8 cores run one identical SPMD instruction stream and differ
only in input data.  Per-edge-tile work: gather k/v rows by src (indirect
DMA), ke = ef@We on PE (+K via identity-matmul PSUM accumulation),
Qe = onehot^T @ q_win on PE, logits via DVE mul + segmented reduce,
w = exp on ACT (broadcast to head width), WV = w*V on DVE, and segment
reduction into the window's node rows via onehot matmuls accumulating in
PSUM.  Window finalize computes sigmoid(relu(agg/denom)@Wd+bd) and scatters
rows to the local output; the host concatenates the 8 node ranges.
"""

import numpy as np
import ml_dtypes

import concourse.bass as bass
import concourse.bacc as bacc
import concourse.mybir as mybir
import concourse.tile as tile

BF16 = ml_dtypes.bfloat16

H, DH = 8, 64
DOUT = H * DH  # 512
N_CORES = 8
TPW = 8  # edge tiles per window
GRP = 4  # tiles gathered per indirect-DMA group


# ----------------------------------------------------------------------------
# Host-side planning
# ----------------------------------------------------------------------------

def make_plan(src, dst, n_nodes, n_cores, tpw):
    E = src.shape[0]
    perm = np.argsort(dst, kind="stable")
    s_src = src[perm]
    s_dst = dst[perm]
    deg = np.bincount(dst, minlength=n_nodes)
    cum = np.concatenate([[0], np.cumsum(deg)])

    cuts = [0]
    for c in range(1, n_cores):
        target = c * E / n_cores
        n = int(np.searchsorted(cum, target))
        n = max(cuts[-1] + 1, min(n, n_nodes - (n_cores - c)))
        cuts.append(n)
    cuts.append(n_nodes)

    cores = []
    for c in range(n_cores):
        nlo, nhi = cuts[c], cuts[c + 1]
        wins = []
        n = nlo
        while n < nhi:
            n2 = n
            edges = 0
            while n2 < nhi and (n2 - n) < 128:
                if edges + deg[n2] > tpw * 128:
                    break
                edges += deg[n2]
                n2 += 1
            assert n2 > n, f"node {n} degree {deg[n]} > {tpw*128}"
            wins.append((n, n2))
            n = n2
        cores.append(dict(nlo=nlo, nhi=nhi, wins=wins))

    NWIN = max(len(c["wins"]) for c in cores)
    QCHUNK = max((c["nhi"] - c["nlo"] + 127) // 128 for c in cores)
    return dict(cores=cores, NWIN=NWIN, QCHUNK=QCHUNK, TPW=tpw,
                s_src=s_src, s_dst=s_dst, perm=perm, cum=cum)


def make_core_inputs(plan, core_idx, ef_sorted, nfT_bf16):
    tpw = plan["TPW"]
    NWIN = plan["NWIN"]
    QCHUNK = plan["QCHUNK"]
    core = plan["cores"][core_idx]
    s_src, cum = plan["s_src"], plan["cum"]
    DE = ef_sorted.shape[1]
    nlo = core["nlo"]
    L = core["nhi"] - nlo
    trash = QCHUNK * 128

    gsrc = np.zeros((NWIN, 128, tpw), np.int32)
    dstloc = np.full((NWIN, 128, tpw), 255.0, BF16)
    efT = np.zeros((NWIN * tpw // GRP, 64, 128 * GRP), BF16)
    dstlocT = np.full((NWIN * tpw // GRP, 128, 128 * GRP), 255.0, BF16)
    wnodes = np.full((NWIN, 128, 1), trash, np.int32)

    for w, (wn_lo, wn_hi) in enumerate(core["wins"]):
        e0, e1 = cum[wn_lo], cum[wn_hi]
        cnt = e1 - e0
        wn = np.arange(wn_lo, wn_hi) - nlo
        wnodes[w, : len(wn), 0] = wn
        sl = np.arange(cnt)
        t_idx = sl // 128
        p_idx = sl % 128
        gsrc[w, p_idx, t_idx] = s_src[e0:e1]
        dl = (plan["s_dst"][e0:e1] - wn_lo).astype(BF16)
        dstloc[w, p_idx, t_idx] = dl
        for t in range(tpw):
            m = t_idx == t
            if not m.any():
                continue
            grp = (w * tpw + t) // GRP
            j = t % GRP
            efT[grp, :DE, j * 128 + p_idx[m]] = ef_sorted[e0:e1][m].astype(BF16)
            dstlocT[grp, :, j * 128 + p_idx[m]] = np.tile(dl[m][:, None], (1, 128))

    nfT_l = np.zeros((QCHUNK, 128, 256), BF16)
    nhi = core["nhi"]
    for i in range(QCHUNK):
        a = nlo + i * 128
        b = min(a + 128, nhi)
        if b > a:
            blk = nfT_bf16[:, a:b]
            nfT_l[i, :, 0 : b - a] = blk[:128]
            nfT_l[i, :, 128 : 128 + b - a] = blk[128:256]
    return dict(gsrc=gsrc, dstloc=dstloc, efT=efT, dstlocT=dstlocT,
                wnodes=wnodes, nfT_l=nfT_l, L=L, nlo=nlo)


def make_global_inputs(nf, Wq, Wk, Wv, We, Wd):
    N, DIN = nf.shape
    nfT = nf.T.astype(BF16)
    NCHUNK = (N + 127) // 128
    nfT_g = np.zeros((NCHUNK, 128, 256), BF16)
    for i in range(NCHUNK):
        a, b = i * 128, min(i * 128 + 128, N)
        nfT_g[i, :, 0 : b - a] = nfT[:128, a:b]
        nfT_g[i, :, 128 : 128 + b - a] = nfT[128:256, a:b]
    scale = 1.0 / np.sqrt(DH)

    def pack_w(W):
        return np.concatenate([W[:128], W[128:256]], axis=1).astype(BF16)

    we_p = np.zeros((64, DOUT), BF16)
    we_p[: We.shape[0]] = We.astype(BF16)
    return dict(
        nfT_g=nfT_g,
        wq=pack_w(Wq * scale),
        wk=pack_w(Wk),
        wv=pack_w(Wv),
        we=we_p,
        wdrow=np.tile(Wd.reshape(1, DOUT), (128, 1)).astype(BF16),
        ident=np.eye(128, dtype=BF16),
        iota_rows=np.tile(np.arange(128, dtype=BF16)[None, :], (128, 1)),
        iota_col=np.arange(128, dtype=BF16).reshape(128, 1),
        nfT=nfT,
        NCHUNK=NCHUNK,
        N=N,
    )


# ----------------------------------------------------------------------------
# Device kernel emission (identical instruction stream on every core)
# ----------------------------------------------------------------------------

def build_nc(N, NCHUNK, NWIN, tpw, QCHUNK, bd0, dbg=False):
    dt = mybir.dt
    bf16, f32, i32 = dt.bfloat16, dt.float32, dt.int32
    NGRP = NWIN * tpw // GRP
    YROWS = QCHUNK * 128 + 128

    nc = bacc.Bacc("TRN2", target_bir_lowering=False, debug=False)

    t_nfT_g = nc.dram_tensor("nfT_g", [NCHUNK, 128, 256], bf16, kind="ExternalInput")
    t_nfT_l = nc.dram_tensor("nfT_l", [QCHUNK, 128, 256], bf16, kind="ExternalInput")
    t_wq = nc.dram_tensor("wq", [128, 2 * DOUT], bf16, kind="ExternalInput")
    t_wk = nc.dram_tensor("wk", [128, 2 * DOUT], bf16, kind="ExternalInput")
    t_wv = nc.dram_tensor("wv", [128, 2 * DOUT], bf16, kind="ExternalInput")
    t_we = nc.dram_tensor("we", [64, DOUT], bf16, kind="ExternalInput")
    t_wdrow = nc.dram_tensor("wdrow", [128, DOUT], bf16, kind="ExternalInput")
    t_ident = nc.dram_tensor("ident", [128, 128], bf16, kind="ExternalInput")
    t_iota_rows = nc.dram_tensor("iota_rows", [128, 128], bf16, kind="ExternalInput")
    t_iota_col = nc.dram_tensor("iota_col", [128, 1], bf16, kind="ExternalInput")
    t_gsrc = nc.dram_tensor("gsrc", [NWIN, 128, tpw], i32, kind="ExternalInput")
    t_dstloc = nc.dram_tensor("dstloc", [NWIN, 128, tpw], bf16, kind="ExternalInput")
    t_dstlocT = nc.dram_tensor("dstlocT", [NGRP, 128, 128 * GRP], bf16, kind="ExternalInput")
    t_efT = nc.dram_tensor("efT", [NGRP, 64, 128 * GRP], bf16, kind="ExternalInput")
    t_wnodes = nc.dram_tensor("wnodes", [NWIN, 128, 1], i32, kind="ExternalInput")

    t_y = nc.dram_tensor("y_out", [YROWS, 1], f32, kind="ExternalOutput")
    t_dbg = {}
    if dbg:
        for nm, shp in [("d_k4", [128, 2 * DOUT]),
                        ("d_ohe", [128, 128]), ("d_ohT", [128, 128]),
                        ("d_kke", [128, DOUT]), ("d_qe", [128, DOUT]),
                        ("d_logits", [128, H]), ("d_wbig", [128, DOUT]),
                        ("d_qwin", [128, DOUT]),
                        ("d_den", [128, H]), ("d_xnorm", [128, DOUT])]:
            t_dbg[nm] = nc.dram_tensor(nm, shp, f32, kind="ExternalOutput")

    def dump(nm, ap):
        if dbg:
            nc.gpsimd.dma_start(out=t_dbg[nm][: ap.shape[0]], in_=ap)

    t_kv = nc.dram_tensor("kv_table", [N, 2 * DOUT], bf16, kind="Internal")
    t_qt = nc.dram_tensor("q_table", [YROWS, DOUT], bf16, kind="Internal")

    with tile.TileContext(nc, pool_alloc_mode="queue") as tc:
        with tc.tile_pool(name="wpool", bufs=1) as wpool:
            wq_sb = wpool.tile([128, 2 * DOUT], bf16)
            nc.sync.dma_start(out=wq_sb[:], in_=t_wq[:])
            wk_sb = wpool.tile([128, 2 * DOUT], bf16)
            nc.sync.dma_start(out=wk_sb[:], in_=t_wk[:])
            wv_sb = wpool.tile([128, 2 * DOUT], bf16)
            nc.sync.dma_start(out=wv_sb[:], in_=t_wv[:])
            we_sb = wpool.tile([64, DOUT], bf16)
            nc.sync.dma_start(out=we_sb[:], in_=t_we[:])
            wdrow_sb = wpool.tile([128, DOUT], bf16)
            nc.sync.dma_start(out=wdrow_sb[:], in_=t_wdrow[:])
            ident_sb = wpool.tile([128, 128], bf16)
            nc.sync.dma_start(out=ident_sb[:], in_=t_ident[:])
            iota_rows_sb = wpool.tile([128, 128], bf16)
            nc.sync.dma_start(out=iota_rows_sb[:], in_=t_iota_rows[:])
            iota_col_sb = wpool.tile([128, 1], bf16)
            nc.sync.dma_start(out=iota_col_sb[:], in_=t_iota_col[:])

            # ---------------- phase 1: k/v tables (all nodes) ----------------
            table_writes = []
            with tc.tile_pool(name="p1", bufs=8) as p1, \
                 tc.tile_pool(name="p1ps", bufs=2, space="PSUM") as p1ps:
                for i in range(NCHUNK):
                    m = min(128, N - i * 128)
                    xt = p1.tile([128, 256], bf16, tag="xt")
                    nc.sync.dma_start(out=xt[:], in_=t_nfT_g[i])
                    ps_kv = p1ps.tile([128, 2 * DOUT], f32, tag="k")
                    for c in range(2):
                        nc.tensor.matmul(ps_kv[:m, :DOUT], xt[:, c * 128 : c * 128 + m], wk_sb[:, c * DOUT : (c + 1) * DOUT],
                                         start=(c == 0), stop=(c == 1))
                        nc.tensor.matmul(ps_kv[:m, DOUT:], xt[:, c * 128 : c * 128 + m], wv_sb[:, c * DOUT : (c + 1) * DOUT],
                                         start=(c == 0), stop=(c == 1))
                    kv_sb = p1.tile([128, 2 * DOUT], bf16, tag="ksb")
                    nc.vector.tensor_copy(kv_sb[:m, :DOUT], ps_kv[:m, :DOUT])
                    nc.scalar.copy(kv_sb[:m, DOUT:], ps_kv[:m, DOUT:])
                    table_writes.append(nc.sync.dma_start(
                        out=t_kv[i * 128 : i * 128 + m, :], in_=kv_sb[:m]))
                # ---------------- phase 1b: q table (local nodes) ----------
                for i in range(QCHUNK):
                    xt = p1.tile([128, 256], bf16, tag="xt")
                    nc.sync.dma_start(out=xt[:], in_=t_nfT_l[i])
                    ps_q = p1ps.tile([128, DOUT], f32, tag="k")
                    for c in range(2):
                        nc.tensor.matmul(ps_q[:], xt[:, c * 128 : (c + 1) * 128], wq_sb[:, c * DOUT : (c + 1) * DOUT],
                                         start=(c == 0), stop=(c == 1))
                    q_sb = p1.tile([128, DOUT], bf16, tag="ksb")
                    nc.vector.tensor_copy(q_sb[:], ps_q[:])
                    table_writes.append(nc.sync.dma_start(
                        out=t_qt[i * 128 : (i + 1) * 128, :], in_=q_sb[:]))
                # zero the trash pad block (gathered by padded window slots)
                zq = p1.tile([128, DOUT], bf16, tag="ksb")
                nc.gpsimd.memset(zq[:], 0)
                table_writes.append(nc.sync.dma_start(
                    out=t_qt[QCHUNK * 128 :, :], in_=zq[:]))

            # Pool-side fence: the gathers are the only table readers and all
            # issue from the Pool sequencer.  Route the fan-in of table-write
            # completion waits into one Pool compute op so no gather DMA ends
            # up with more waits than the DMA lowering allows.
            fence_tile = wpool.tile([1, 4], mybir.dt.int32)
            fence = nc.gpsimd.memset(fence_tile[:], 0)
            for wdma in table_writes:
                tile.add_dep_helper(fence.ins, wdma.ins, sync=True,
                                    reason="table fence")

            # ---------------- phase 2: edge phase ----------------
            with tc.tile_pool(name="p2", bufs=4) as p2, \
                 tc.tile_pool(name="p2s", bufs=8) as p2s, \
                 tc.tile_pool(name="p2w", bufs=2) as p2w, \
                 tc.tile_pool(name="psA", bufs=2, space="PSUM") as psA, \
                 tc.tile_pool(name="psB", bufs=2, space="PSUM") as psB:
                for w in range(NWIN):
                    widx = p2w.tile([128, 1], i32, tag="widx")
                    nc.sync.dma_start(out=widx[:], in_=t_wnodes[w])
                    widx_y = p2w.tile([128, 1], i32, tag="widx_y")
                    nc.sync.dma_start(out=widx_y[:], in_=t_wnodes[w])
                    qwin = p2w.tile([128, DOUT], bf16, tag="qwin")
                    nc.gpsimd.indirect_dma_start(
                        out=qwin[:], out_offset=None, in_=t_qt[:],
                        in_offset=bass.IndirectOffsetOnAxis(ap=widx[:, :1], axis=0))
                    dloc = p2w.tile([128, tpw], bf16, tag="dloc")
                    nc.sync.dma_start(out=dloc[:], in_=t_dstloc[w])
                    idx_w = p2w.tile([128, tpw], i32, tag="idx_w")
                    nc.sync.dma_start(out=idx_w[:], in_=t_gsrc[w])
                    agg = psA.tile([128, DOUT], f32, tag="agg")
                    den = psA.tile([128, H], f32, tag="den")
                    for g in range(tpw // GRP):
                        grp = w * (tpw // GRP) + g
                        efT4 = p2.tile([64, 128 * GRP], bf16, tag="efT4")
                        nc.sync.dma_start(out=efT4[:], in_=t_efT[grp])
                        dT4 = p2.tile([128, 128 * GRP], bf16, tag="dT4", bufs=8)
                        nc.sync.dma_start(out=dT4[:], in_=t_dstlocT[grp])
                        for j in range(GRP):
                            t = g * GRP + j
                            kvg = p2.tile([128, 2 * DOUT], bf16, tag="kvg")
                            nc.gpsimd.indirect_dma_start(
                                out=kvg[:], out_offset=None, in_=t_kv[:],
                                in_offset=bass.IndirectOffsetOnAxis(ap=idx_w[:, t : t + 1], axis=0))
                            k_j = kvg[:, :DOUT]
                            v_j = kvg[:, DOUT:]
                            ps_ke = psB.tile([128, DOUT], f32, tag="ke")
                            nc.tensor.matmul(ps_ke[:], efT4[:, j * 128 : (j + 1) * 128],
                                             we_sb[:], start=True, stop=False)
                            nc.tensor.matmul(ps_ke[:], ident_sb[:], k_j,
                                             start=False, stop=True)
                            kke = p2.tile([128, DOUT], bf16, tag="kke")
                            nc.scalar.copy(kke[:], ps_ke[:])
                            oh_e = p2.tile([128, 128], bf16, tag="oh_e")
                            nc.vector.tensor_tensor(
                                oh_e[:], dloc[:, t : t + 1].to_broadcast([128, 128]),
                                iota_rows_sb[:], mybir.AluOpType.is_equal)
                            oh_T = p2.tile([128, 128], bf16, tag="oh_T")
                            nc.vector.tensor_tensor(
                                oh_T[:], iota_col_sb[:].to_broadcast([128, 128]),
                                dT4[:, j * 128 : (j + 1) * 128],
                                mybir.AluOpType.is_equal)
                            ps_qe = psB.tile([128, DOUT], f32, tag="qe")
                            nc.tensor.matmul(ps_qe[:], oh_T[:], qwin[:],
                                             start=True, stop=True)
                            if dbg and w == 0 and t == 0:
                                dump("d_k4", kvg[:])
                                dump("d_ohe", oh_e[:]); dump("d_ohT", oh_T[:])
                                dump("d_kke", kke[:]); dump("d_qwin", qwin[:])
                            qe = p2.tile([128, DOUT], bf16, tag="qe_sb")
                            nc.scalar.copy(qe[:], ps_qe[:])
                            prod = p2.tile([128, DOUT], bf16, tag="prod")
                            nc.vector.tensor_tensor(prod[:], qe[:], kke[:],
                                                    mybir.AluOpType.mult)
                            logits = p2.tile([128, H], f32, tag="logits")
                            nc.vector.tensor_reduce(
                                logits[:], prod[:].rearrange("p (h d) -> p h d", h=H),
                                mybir.AxisListType.X, mybir.AluOpType.add)
                            wbig = p2.tile([128, H], bf16, tag="wbig")
                            nc.scalar.activation(wbig[:], logits[:],
                                mybir.ActivationFunctionType.Exp)
                            if dbg and w == 0 and t == 0:
                                dump("d_qe", qe[:]); dump("d_logits", logits[:])
                                dump("d_wbig", wbig[:])
                            wv_t = p2.tile([128, DOUT], bf16, tag="wv")
                            nc.vector.tensor_tensor(
                                wv_t[:].rearrange("p (h d) -> p h d", h=H),
                                wbig[:, :, None].to_broadcast([128, H, DH]),
                                v_j.rearrange("p (h d) -> p h d", h=H),
                                mybir.AluOpType.mult)
                            nc.tensor.matmul(agg[:], oh_e[:], wv_t[:],
                                             start=(t == 0), stop=(t == tpw - 1))
                            nc.tensor.matmul(den[:], oh_e[:], wbig[:],
                                             start=(t == 0), stop=(t == tpw - 1))
                    den_sb = p2w.tile([128, H], f32, tag="den_sb")
                    nc.vector.tensor_scalar_add(den_sb[:], den[:], 1e-9)
                    recip = p2w.tile([128, H], f32, tag="recip")
                    nc.vector.reciprocal(recip[:], den_sb[:])
                    xnorm = p2w.tile([128, DOUT], bf16, tag="xnorm")
                    nc.vector.tensor_tensor(
                        xnorm[:].rearrange("p (h d) -> p h d", h=H),
                        agg[:].rearrange("p (h d) -> p h d", h=H),
                        recip[:, :, None].to_broadcast([128, H, DH]),
                        mybir.AluOpType.mult)
                    if dbg and w == 0:
                        dump("d_den", den_sb[:])
                        dump("d_xnorm", xnorm[:])
                    scr = p2w.tile([128, DOUT], bf16, tag="scr")
                    ypre = p2w.tile([128, 1], f32, tag="ypre")
                    nc.vector.scalar_tensor_tensor(
                        out=scr[:], in0=xnorm[:], scalar=0.0, in1=wdrow_sb[:],
                        op0=mybir.AluOpType.max, op1=mybir.AluOpType.mult,
                        accum_out=ypre[:])
                    y_sb = p2w.tile([128, 1], f32, tag="y_sb")
                    nc.scalar.activation(y_sb[:], ypre[:],
                                         mybir.ActivationFunctionType.Sigmoid,
                                         bias=float(bd0))
                    nc.gpsimd.indirect_dma_start(
                        out=t_y[:], out_offset=bass.IndirectOffsetOnAxis(
                            ap=widx_y[:, :1], axis=0),
                        in_=y_sb[:], in_offset=None)
    nc.compile()
    return nc


# ----------------------------------------------------------------------------
# Entry point
# ----------------------------------------------------------------------------

LAST_RESULTS = None  # BassKernelResults of the most recent run (for profiling)
LAST_NC = None


def kernel(node_features, edge_features, Wq, Wk, Wv, We, Wd, bd, src, dst,
           trace=False):
    from concourse.bass_utils import run_bass_kernel_spmd

    nf = np.asarray(node_features, dtype=np.float32)
    ef = np.asarray(edge_features, dtype=np.float32)
    src = np.asarray(src, dtype=np.int32)
    dst = np.asarray(dst, dtype=np.int32)
    Wq = np.asarray(Wq, np.float32)
    Wk = np.asarray(Wk, np.float32)
    Wv = np.asarray(Wv, np.float32)
    We = np.asarray(We, np.float32)
    Wd = np.asarray(Wd, np.float32)
    bd = np.asarray(bd, np.float32)
    N = nf.shape[0]

    plan = make_plan(src, dst, N, N_CORES, TPW)
    gin = make_global_inputs(nf, Wq, Wk, Wv, We, Wd)
    ef_sorted = ef[plan["perm"]]

    nc = build_nc(N=gin["N"], NCHUNK=gin["NCHUNK"], NWIN=plan["NWIN"],
                  tpw=TPW, QCHUNK=plan["QCHUNK"], bd0=float(bd.ravel()[0]))

    shared = {k: gin[k] for k in ("nfT_g", "wq", "wk", "wv", "we", "wdrow",
                                  "ident", "iota_rows", "iota_col")}
    in_maps = []
    core_meta = []
    for c in range(N_CORES):
        cin = make_core_inputs(plan, c, ef_sorted, gin["nfT"])
        m = dict(shared)
        for k in ("nfT_l", "gsrc", "dstloc", "dstlocT", "efT", "wnodes"):
            m[k] = cin[k]
        in_maps.append(m)
        core_meta.append((cin["nlo"], cin["L"]))

    res = run_bass_kernel_spmd(nc, in_maps, core_ids=list(range(N_CORES)),
                               trace=trace)
    global LAST_RESULTS, LAST_NC
    LAST_RESULTS = res
    LAST_NC = nc

    y = np.zeros((N, 1), np.float32)
    for c, (nlo, L) in enumerate(core_meta):
        y[nlo : nlo + L, 0] = res.results[c]["y_out"][:L, 0]
    return y



# revision 21
# speedup vs baseline: 1.7293x; 1.7293x over previous
"""GAT message-passing model on 8 Trainium2 NeuronCores.

Strategy (v3): edges sorted by destination on the host; nodes split into 8
contiguous ranges balanced by incoming-edge count (one per core).  Each HBM
core-pair builds ONE shared k/v node table (addr_space="Shared"): the even
core writes table rows [0,25088), the odd core [25088,50176), via direct
DMAs whose row offset comes from a partition_id()-derived register, then a
tiny pairwise AllGather acts as the cross-core barrier.  Per-window q
projections stay resident in SBUF.  The edge phase processes windows of
<=128 dst nodes / <=1024 edges: one fused input-stream DMA per window
(one-hots both orientations, transposed edge features, gather indices),
one indirect gather per window for all 8 edge-tiles' k|v rows; per tile
the kernel computes kkeT = We^T@efT + k^T (PE, identity-matmul transpose
trick), qeT via host-precomputed one-hots (PE), prodT = qeT*kkeT (DVE,
straight from both PSUM banks), per-head logits via head-selector
reduction matmuls (PE), logits un-transposed by a tiny matmul, exp+head-
expansion fused in one ACT op, w*v on DVE (2x mode), and segment-sums into
PSUM via one-hot matmuls.  Window finalize normalizes agg/den and dots
with Wd into a per-window column of an SBUF accumulator; one sigmoid + one
DMA at the very end write all window slots, which the host scatters back
to node order.
"""

import numpy as np
import ml_dtypes

import concourse.bass as bass
import concourse.bacc as bacc
import concourse.mybir as mybir
import concourse.tile as tile

BF16 = ml_dtypes.bfloat16

N_NODES = 50000
H, DH = 8, 64
DOUT = H * DH  # 512
N_CORES = 8
TPW = 8           # edge tiles per window
PAIR_SHARED = True
HLEN = 25088      # 196 chunks of 128; rows 50000..50175 are zero pads
TRASH = 50000     # zero row of the table (odd half's pad region)

# packed constant layout (columns of the single wconst input)
C_WQ = 0
C_WK = C_WQ + 2 * DOUT
C_WV = C_WK + 2 * DOUT
C_WD = C_WV + 2 * DOUT
C_ID = C_WD + DOUT
C_WE = C_ID + 128
C_ID8 = C_WE + DOUT
C_HS = C_ID8 + 8
C_END = C_HS + 4 * 8

# fused per-window stream layout (bf16 columns)
S_OHT = 0                  # [128, 1024]  one-hot [n, e] per tile block
S_OHE = 1024               # [128, 1024]  one-hot [e, n] per tile block
S_EFT = 2048               # [64, 1024]   edge features transposed
S_GS = 3072                # [128, 16]    gather indices (int32 as 2xbf16)
S_END = 3088


# ----------------------------------------------------------------------------
# Host-side planning
# ----------------------------------------------------------------------------

def make_plan(src, dst, n_nodes, n_cores, tpw):
    E = src.shape[0]
    perm = np.argsort(dst, kind="stable")
    s_src = src[perm]
    s_dst = dst[perm]
    deg = np.bincount(dst, minlength=n_nodes)
    cum = np.concatenate([[0], np.cumsum(deg)])

    cuts = [0]
    for c in range(1, n_cores):
        target = c * E / n_cores
        n = int(np.searchsorted(cum, target))
        n = max(cuts[-1] + 1, min(n, n_nodes - (n_cores - c)))
        cuts.append(n)
    cuts.append(n_nodes)

    cores = []
    for c in range(n_cores):
        nlo, nhi = cuts[c], cuts[c + 1]
        wins = []
        n = nlo
        while n < nhi:
            n2 = n
            edges = 0
            while n2 < nhi and (n2 - n) < 128:
                if edges + deg[n2] > tpw * 128:
                    break
                edges += deg[n2]
                n2 += 1
            assert n2 > n, f"node {n} degree {deg[n]} > {tpw*128}"
            wins.append((n, n2))
            n = n2
        cores.append(dict(nlo=nlo, nhi=nhi, wins=wins))

    NWIN = max(len(c["wins"]) for c in cores)
    return dict(cores=cores, NWIN=NWIN, s_src=s_src, s_dst=s_dst,
                perm=perm, cum=cum)


def make_core_inputs(plan, core_idx, ef_sorted, nfT):
    """Per-core edge-phase inputs: fused stream + per-window q features."""
    NWIN = plan["NWIN"]
    core = plan["cores"][core_idx]
    cum = plan["cum"]
    DE = ef_sorted.shape[1]
    NW4 = (NWIN + 3) // 4

    strm = np.zeros((NWIN, 128, S_END), BF16)
    gsrc = np.full((NWIN, 128, TPW), TRASH, np.int32)
    nfT_w = np.zeros((NW4, 128, 4 * 256), BF16)

    for w, (a, b) in enumerate(core["wins"]):
        e0, e1 = cum[a], cum[b]
        cnt = e1 - e0
        sl = np.arange(cnt)
        t_idx = sl // 128
        p_idx = sl % 128
        gsrc[w, p_idx, t_idx] = plan["s_src"][e0:e1]
        dl = (plan["s_dst"][e0:e1] - a).astype(np.int64)
        strm[w, dl, S_OHT + t_idx * 128 + p_idx] = 1
        strm[w, p_idx, S_OHE + t_idx * 128 + dl] = 1
        strm[w, :DE, S_EFT + t_idx * 128 + p_idx] = \
            ef_sorted[e0:e1].astype(BF16)
        L = b - a
        nfT_w[w // 4, :, (w % 4) * 256:(w % 4) * 256 + L] = nfT[:128, a:b]
        nfT_w[w // 4, :, (w % 4) * 256 + 128:(w % 4) * 256 + 128 + L] = \
            nfT[128:256, a:b]
    strm[:, :, S_GS:S_END] = gsrc.view(np.uint16).view(BF16).reshape(
        NWIN, 128, 16)
    return dict(strm=strm, nfT_w=nfT_w, wins=core["wins"])


def make_table_inputs(core_idx, nfT, n_nodes, pair_shared):
    """Table-build node-feature chunks (x4-fused): this core's rows."""
    if pair_shared:
        half = core_idx % 2
        r0 = half * HLEN
        r1 = min(r0 + HLEN, n_nodes)
        hchunk = HLEN // 128
    else:
        r0, r1 = 0, n_nodes
        hchunk = 2 * HLEN // 128
    assert hchunk % 4 == 0
    nfT_h = np.zeros((hchunk // 4, 128, 4 * 256), BF16)
    for i in range(hchunk):
        a = r0 + i * 128
        b = min(a + 128, r1)
        if b > a:
            L = b - a
            c0 = (i % 4) * 256
            nfT_h[i // 4, :, c0:c0 + L] = nfT[:128, a:b]
            nfT_h[i // 4, :, c0 + 128:c0 + 128 + L] = nfT[128:256, a:b]
    return dict(nfT_h=nfT_h, HCHUNK=hchunk)


def make_global_inputs(nf, Wq, Wk, Wv, We, Wd):
    N, DIN = nf.shape
    nfT = nf.T.astype(BF16)
    scale = 1.0 / np.sqrt(DH)

    def pack_w(W):
        return np.concatenate([W[:128], W[128:256]], axis=1).astype(BF16)

    wconst = np.zeros((128, C_END), BF16)
    wconst[:, C_WQ:C_WQ + 2 * DOUT] = pack_w(Wq * scale)
    wconst[:, C_WK:C_WK + 2 * DOUT] = pack_w(Wk)
    wconst[:, C_WV:C_WV + 2 * DOUT] = pack_w(Wv)
    wconst[:, C_WD:C_WD + DOUT] = np.tile(Wd.reshape(1, DOUT), (128, 1))
    wconst[:, C_ID:C_ID + 128] = np.eye(128)
    wconst[:64, C_WE:C_WE + DOUT] = We.astype(BF16)
    wconst[:8, C_ID8:C_ID8 + 8] = np.eye(8)
    # hsel[c][d, h] = 1 iff head h == 2c + (d >= 64): reduction selectors
    # that drop chunk c's two heads into rows 2c / 2c+1 of the logits PSUM.
    for c in range(4):
        wconst[:64, C_HS + c * 8 + 2 * c] = 1
        wconst[64:, C_HS + c * 8 + 2 * c + 1] = 1
    return dict(wconst=wconst, nfT=nfT, N=N)


# ----------------------------------------------------------------------------
# Device kernel emission (identical instruction stream on every core)
# ----------------------------------------------------------------------------

def build_nc(NWIN, HCHUNK, bd0, pair_shared=PAIR_SHARED):
    import os
    STAGE = int(os.environ.get("KSTAGE", "3"))  # 1=table,2=+q,3=full
    dt = mybir.dt
    bf16, f32, i32 = dt.bfloat16, dt.float32, dt.int32
    NROW = 2 * HLEN + 128
    NW4 = (NWIN + 3) // 4
    HB = HCHUNK // 4

    nc = bacc.Bacc("TRN2", target_bir_lowering=False, debug=False)

    t_wc = nc.dram_tensor("wconst", [128, C_END], bf16, kind="ExternalInput")
    t_nfT_h = nc.dram_tensor("nfT_h", [HB, 128, 4 * 256], bf16, kind="ExternalInput")
    t_nfT_w = nc.dram_tensor("nfT_w", [NW4, 128, 4 * 256], bf16, kind="ExternalInput")
    t_strm = nc.dram_tensor("strm", [NWIN, 128, S_END], bf16, kind="ExternalInput")

    t_y = nc.dram_tensor("y_out", [128, NWIN], f32, kind="ExternalOutput")
    DBG = int(os.environ.get("KDBG", "0"))
    t_dbg = {}
    if DBG:
        for nm, shp, dtp in [("d_kvw", [128, 2048], bf16),
                             ("d_kke", [128, 1024], bf16),
                             ("d_prod", [128, 1024], bf16),
                             ("d_wT", [8, 256], bf16),
                             ("d_wsb", [128, 16], bf16),
                             ("d_wv", [128, 512], bf16),
                             ("d_q", [128, 512], bf16),
                             ("d_aggsb", [128, 512], bf16),
                             ("d_xn", [128, 512], bf16)]:
            t_dbg[nm] = nc.dram_tensor(nm, shp, dtp, kind="ExternalOutput")

    t_kv = nc.dram_tensor("kv_table", [NROW, 2 * DOUT], bf16, kind="Internal",
                          addr_space="Shared" if pair_shared else "Local")
    if pair_shared:
        t_cc_in = nc.dram_tensor("cc_in", [1, 4], i32, kind="Internal")
        t_cc_out = nc.dram_tensor("cc_out", [2, 4], i32, kind="Internal")

    with tile.TileContext(nc, pool_alloc_mode="queue") as tc:
        with tc.tile_pool(name="wpool", bufs=1) as wpool:
            wc = wpool.tile([128, C_END], bf16)
            nc.sync.dma_start(out=wc[:], in_=t_wc[:])
            wq_sb = wc[:, C_WQ:C_WQ + 2 * DOUT]
            wk_sb = wc[:, C_WK:C_WK + 2 * DOUT]
            wv_sb = wc[:, C_WV:C_WV + 2 * DOUT]
            wdrow_sb = wc[:, C_WD:C_WD + DOUT]
            ident_sb = wc[:, C_ID:C_ID + 128]
            we_sb = wc[:64, C_WE:C_WE + DOUT]
            ident8_sb = wc[:8, C_ID8:C_ID8 + 8]
            hsel_sb = [wc[:, C_HS + c * 8:C_HS + (c + 1) * 8] for c in range(4)]
            q_all = wpool.tile([128, NWIN * DOUT], bf16)
            y_acc = wpool.tile([128, NWIN], f32)

            # slot base register: pair half = partition_id() % 2
            if pair_shared:
                pid = nc.sync.partition_id()
                base = (pid % 2) * HB
            else:
                base = 0

            # ---------------- phase 1: k/v table (this core's share) --------
            table_writes = []
            with tc.tile_pool(name="p1", bufs=4) as p1, \
                 tc.tile_pool(name="p1ps", bufs=2, space="PSUM") as p1ps:
                for i4 in range(HB):
                    xt = p1.tile([128, 4 * 256], bf16, tag="xt")
                    nc.scalar.dma_start(out=xt[:], in_=t_nfT_h[i4])
                    kv4 = p1.tile([128, 4 * 2 * DOUT], bf16, tag="kv4")
                    for s in range(4):
                        ps_kv = p1ps.tile([128, 2 * DOUT], f32, tag="kv", bufs=3)
                        for c in range(2):
                            nc.tensor.matmul(
                                ps_kv[:, :DOUT],
                                xt[:, s * 256 + c * 128:s * 256 + (c + 1) * 128],
                                wk_sb[:, c * DOUT:(c + 1) * DOUT],
                                start=(c == 0), stop=(c == 1))
                            nc.tensor.matmul(
                                ps_kv[:, DOUT:],
                                xt[:, s * 256 + c * 128:s * 256 + (c + 1) * 128],
                                wv_sb[:, c * DOUT:(c + 1) * DOUT],
                                start=(c == 0), stop=(c == 1))
                        o = s * 2 * DOUT
                        nc.vector.tensor_copy(kv4[:, o:o + DOUT],
                                              ps_kv[:, :DOUT])
                        nc.scalar.copy(kv4[:, o + DOUT:o + 2 * DOUT],
                                       ps_kv[:, DOUT:])
                    dst = t_kv[bass.ts(base + i4, 512)].rearrange(
                        "(c p) e -> p c e", p=128)
                    table_writes.append(
                        nc.sync.dma_start(out=dst, in_=kv4[:].rearrange(
                            "p (c e) -> p c e", c=4)))

                # ---------------- phase 1b: per-window q (stays in SBUF) ----
                for w4 in range(NW4 if STAGE >= 2 else 0):
                    xt = p1.tile([128, 4 * 256], bf16, tag="xt")
                    nc.sync.dma_start(out=xt[:], in_=t_nfT_w[w4])
                    for s in range(4):
                        w = w4 * 4 + s
                        if w >= NWIN:
                            break
                        ps_q = p1ps.tile([128, DOUT], f32, tag="q", bufs=2)
                        for c in range(2):
                            nc.tensor.matmul(
                                ps_q[:],
                                xt[:, s * 256 + c * 128:s * 256 + (c + 1) * 128],
                                wq_sb[:, c * DOUT:(c + 1) * DOUT],
                                start=(c == 0), stop=(c == 1))
                        if w % 2 == 0:
                            nc.vector.tensor_copy(
                                q_all[:, w * DOUT:(w + 1) * DOUT], ps_q[:])
                        else:
                            nc.scalar.copy(
                                q_all[:, w * DOUT:(w + 1) * DOUT], ps_q[:])

            # Fence: all table writes must complete before any edge gather.
            fence_tile = wpool.tile([1, 4], i32)
            fence = nc.gpsimd.memset(fence_tile[:], 0)
            for wdma in table_writes:
                tile.add_dep_helper(fence.ins, wdma.ins, sync=True,
                                    reason="table fence")
            if pair_shared:
                # tiny pairwise AllGather = cross-core barrier for the pair
                cc_seed = wpool.tile([1, 4], i32)
                nc.gpsimd.memset(cc_seed[:], 0)
                seed_dma = nc.sync.dma_start(out=t_cc_in[:], in_=cc_seed[:])
                cc = nc.gpsimd.collective_compute(
                    kind="AllGather",
                    op=mybir.AluOpType.bypass,
                    replica_groups=[[0, 1], [2, 3], [4, 5], [6, 7]],
                    ins=[t_cc_in[:]],
                    outs=[t_cc_out[:]],
                )
                tile.add_dep_helper(cc.ins, fence.ins, sync=True,
                                    reason="barrier after table")
                tile.add_dep_helper(cc.ins, seed_dma.ins, sync=True,
                                    reason="barrier seed")
                gate = nc.gpsimd.memset(fence_tile[:], 1)
                tile.add_dep_helper(gate.ins, cc.ins, sync=True,
                                    reason="gate on barrier")
            else:
                gate = fence

            # ---------------- phase 2: edge phase ----------------
            with tc.tile_pool(name="p2", bufs=6) as p2, \
                 tc.tile_pool(name="p2s", bufs=3) as p2s, \
                 tc.tile_pool(name="p2w", bufs=2) as p2w, \
                 tc.tile_pool(name="psK", bufs=1, space="PSUM") as psK, \
                 tc.tile_pool(name="psQ", bufs=1, space="PSUM") as psQ, \
                 tc.tile_pool(name="psL", bufs=2, space="PSUM") as psL, \
                 tc.tile_pool(name="psA", bufs=1, space="PSUM") as psA:
                for w in range(NWIN if STAGE >= 3 else 0):
                    strm = p2s.tile([128, S_END], bf16, tag="strm")
                    nc.scalar.dma_start(out=strm[:], in_=t_strm[w])
                    kvw = p2w.tile([128, TPW * 2 * DOUT], bf16, tag="kvw")
                    gso = strm[:, S_GS:S_END].bitcast(i32)
                    for t in range(TPW):
                        g_ins = nc.gpsimd.indirect_dma_start(
                            out=kvw[:, t * 2 * DOUT:(t + 1) * 2 * DOUT],
                            out_offset=None, in_=t_kv[:],
                            in_offset=bass.IndirectOffsetOnAxis(
                                ap=gso[:, t:t + 1], axis=0))
                        tile.add_dep_helper(g_ins.ins, gate.ins, sync=True,
                                            reason="wait table barrier")
                    agg = psA.tile([128, DOUT], f32, tag="agg")
                    den = psA.tile([128, H], f32, tag="den")
                    for g in range(TPW // 2):
                        t0 = 2 * g
                        efT_2 = strm[:64, S_EFT + t0 * 128:
                                     S_EFT + (t0 + 2) * 128]
                        ohT_2 = strm[:, S_OHT + t0 * 128:S_OHT + (t0 + 2) * 128]
                        ps_k = psK.tile([128, 2 * DOUT], f32, tag="kke")
                        ps_q = psQ.tile([128, 2 * DOUT], f32, tag="qe")
                        for c in range(4):
                            co = c * 256
                            nc.tensor.matmul(
                                ps_k[:, co:co + 256],
                                we_sb[:, c * 128:(c + 1) * 128], efT_2,
                                start=True, stop=False)
                            for u in range(2):
                                t = t0 + u
                                nc.tensor.matmul(
                                    ps_k[:, co + u * 128:co + (u + 1) * 128],
                                    kvw[:, t * 2 * DOUT + c * 128:
                                        t * 2 * DOUT + (c + 1) * 128],
                                    ident_sb, start=False, stop=True)
                            nc.tensor.matmul(
                                ps_q[:, co:co + 256],
                                q_all[:, w * DOUT + c * 128:
                                      w * DOUT + (c + 1) * 128],
                                ohT_2, start=True, stop=True)
                        kke_sb = p2.tile([128, 2 * DOUT], bf16, tag="kke_sb")
                        nc.scalar.copy(kke_sb[:], ps_k[:])
                        if DBG and w == 0 and g == 0:
                            nc.gpsimd.dma_start(out=t_dbg["d_kvw"][:], in_=kvw[:, :2048])
                            nc.gpsimd.dma_start(out=t_dbg["d_kke"][:], in_=kke_sb[:])
                            nc.gpsimd.dma_start(out=t_dbg["d_q"][:], in_=q_all[:, :512])
                        prodT = p2.tile([128, 2 * DOUT], bf16, tag="prodT")
                        nc.vector.tensor_tensor(
                            prodT[:], ps_q[:], kke_sb[:], mybir.AluOpType.mult)
                        lw = psL.tile([128, 272], f32, tag="lw")
                        for c in range(4):
                            nc.tensor.matmul(
                                lw[:8, :256], hsel_sb[c],
                                prodT[:, c * 256:(c + 1) * 256],
                                start=(c == 0), stop=(c == 3))
                        wT_sb = p2.tile([8, 256], bf16, tag="wT")
                        nc.scalar.activation(wT_sb[:], lw[:8, :256],
                                             mybir.ActivationFunctionType.Exp)
                        for u in range(2):
                            nc.tensor.matmul(
                                lw[:, 256 + u * 8:256 + (u + 1) * 8],
                                wT_sb[:, u * 128:(u + 1) * 128], ident8_sb,
                                start=True, stop=True)
                        w_sb = p2.tile([128, 16], bf16, tag="w_sb")
                        nc.scalar.copy(w_sb[:], lw[:, 256:272])
                        if DBG and w == 0 and g == 0:
                            nc.gpsimd.dma_start(out=t_dbg["d_prod"][:], in_=prodT[:])
                            nc.gpsimd.dma_start(out=t_dbg["d_wT"][:], in_=wT_sb[:])
                            nc.gpsimd.dma_start(out=t_dbg["d_wsb"][:], in_=w_sb[:])
                        for u in range(2):
                            t = t0 + u
                            vslice = kvw[:, t * 2 * DOUT + DOUT:
                                         (t + 1) * 2 * DOUT]
                            ohe_t = strm[:, S_OHE + t * 128:
                                         S_OHE + (t + 1) * 128]
                            wv_t = p2.tile([128, DOUT], bf16, tag="wv")
                            if u == 0:
                                wx = p2.tile([128, H, DH], bf16, tag="wx")
                                nc.scalar.copy(
                                    wx[:], w_sb[:, t * 8 - t0 * 8:][:, :H]
                                    [:, :, None].to_broadcast([128, H, DH]))
                                nc.vector.tensor_tensor(
                                    wv_t[:], wx[:].rearrange("p h d -> p (h d)"),
                                    vslice, mybir.AluOpType.mult)
                            else:
                                nc.vector.tensor_tensor(
                                    wv_t[:].rearrange("p (h d) -> p h d", h=H),
                                    w_sb[:, u * 8:(u + 1) * 8][:, :, None]
                                    .to_broadcast([128, H, DH]),
                                    vslice.rearrange("p (h d) -> p h d", h=H),
                                    mybir.AluOpType.mult)
                            if DBG and w == 0 and t == 0:
                                nc.gpsimd.dma_start(out=t_dbg["d_wv"][:], in_=wv_t[:])
                            nc.tensor.matmul(
                                agg[:], ohe_t, wv_t[:],
                                start=(t == 0), stop=(t == TPW - 1))
                            nc.tensor.matmul(
                                den[:], ohe_t, w_sb[:, u * 8:(u + 1) * 8],
                                start=(t == 0), stop=(t == TPW - 1))
                    # window finalize: free agg/den ASAP, all-bf16 fast path
                    agg_sb = p2w.tile([128, DOUT], bf16, tag="agg_sb")
                    nc.scalar.copy(agg_sb[:], agg[:])
                    den_sb = p2w.tile([128, H], f32, tag="den_sb")
                    nc.vector.tensor_scalar_add(den_sb[:], den[:], 1e-9)
                    recip = p2w.tile([128, H], f32, tag="recip")
                    nc.vector.reciprocal(recip[:], den_sb[:])
                    rexp = p2w.tile([128, H, DH], bf16, tag="rexp")
                    nc.vector.tensor_copy(
                        rexp[:], recip[:, :, None].to_broadcast([128, H, DH]))
                    if DBG and w == 0:
                        nc.gpsimd.dma_start(out=t_dbg["d_aggsb"][:], in_=agg_sb[:])
                    xn = p2w.tile([128, DOUT], bf16, tag="xn")
                    nc.vector.tensor_tensor(
                        xn[:], agg_sb[:],
                        rexp[:].rearrange("p h d -> p (h d)"),
                        mybir.AluOpType.mult)
                    if DBG and w == 0:
                        nc.gpsimd.dma_start(out=t_dbg["d_xn"][:], in_=xn[:])
                    scr = p2w.tile([128, DOUT], bf16, tag="scr")
                    nc.vector.scalar_tensor_tensor(
                        out=scr[:], in0=xn[:], scalar=0.0, in1=wdrow_sb,
                        op0=mybir.AluOpType.max, op1=mybir.AluOpType.mult,
                        accum_out=y_acc[:, w:w + 1])
                # one sigmoid + one DMA for all windows
                if STAGE >= 3:
                    y_sb = wpool.tile([128, NWIN], f32)
                    nc.scalar.activation(y_sb[:], y_acc[:],
                                         mybir.ActivationFunctionType.Sigmoid,
                                         bias=float(bd0))
                    nc.sync.dma_start(out=t_y[:], in_=y_sb[:])
    nc.compile()
    return nc


# ----------------------------------------------------------------------------
# Entry point
# ----------------------------------------------------------------------------

LAST_RESULTS = None
LAST_NC = None


def kernel(node_features, edge_features, Wq, Wk, Wv, We, Wd, bd, src, dst,
           trace=False):
    from concourse.bass_utils import run_bass_kernel_spmd

    nf = np.asarray(node_features, dtype=np.float32)
    ef = np.asarray(edge_features, dtype=np.float32)
    src = np.asarray(src, dtype=np.int32)
    dst = np.asarray(dst, dtype=np.int32)
    Wq = np.asarray(Wq, np.float32)
    Wk = np.asarray(Wk, np.float32)
    Wv = np.asarray(Wv, np.float32)
    We = np.asarray(We, np.float32)
    Wd = np.asarray(Wd, np.float32)
    bd = np.asarray(bd, np.float32)
    N = nf.shape[0]

    plan = make_plan(src, dst, N, N_CORES, TPW)
    gin = make_global_inputs(nf, Wq, Wk, Wv, We, Wd)
    ef_sorted = ef[plan["perm"]]

    tin0 = make_table_inputs(0, gin["nfT"], N, PAIR_SHARED)
    HCHUNK = tin0["HCHUNK"]

    nc = build_nc(NWIN=plan["NWIN"], HCHUNK=HCHUNK,
                  bd0=float(bd.ravel()[0]), pair_shared=PAIR_SHARED)

    in_maps = []
    core_meta = []
    for c in range(N_CORES):
        cin = make_core_inputs(plan, c, ef_sorted, gin["nfT"])
        tin = make_table_inputs(c, gin["nfT"], N, PAIR_SHARED)
        m = dict(wconst=gin["wconst"], nfT_h=tin["nfT_h"],
                 strm=cin["strm"], nfT_w=cin["nfT_w"])
        in_maps.append(m)
        core_meta.append(cin["wins"])

    res = run_bass_kernel_spmd(nc, in_maps, core_ids=list(range(N_CORES)),
                               trace=trace)
    global LAST_RESULTS, LAST_NC
    LAST_RESULTS = res
    LAST_NC = nc

    y = np.zeros((N, 1), np.float32)
    for c, wins in enumerate(core_meta):
        yc = res.results[c]["y_out"]
        for w, (a, b) in enumerate(wins):
            y[a:b, 0] = yc[:b - a, w]
    return y
